# revision 15
# baseline (speedup 1.0000x reference)
"""AttentionPool Trainium2 kernel.

Computes, for x [B, N, D], mask [B, N], q [D]:
    logits = einsum('bnd,d->bn', x, q);  logits[~mask] = -inf
    w = softmax(logits, axis=-1)
    out = einsum('bn,bnd->bd', w, x)

Sharding: data-parallel over B across 8 NeuronCores (4 rows per core).

Position enumeration (per row): n = t8*1024 + p*8 + s, with p = SBUF
partition, s in [0,8), t8 in [0,8). Each partition reads 8 consecutive
positions = 8 KiB contiguous DRAM per (p, t8). Column col = t8*8 + s.

v3 design (memory-roofline targeted; measured v2 -> v3 notes inline):
  - x is DMA'd via the SWDGE (gpsimd) path with an inline f32 -> fp16
    cast: HBM reads stay f32 (33.5 MB/core, the roofline; measured
    ~406 GB/s read-side on this part), SBUF holds fp16. This removes the
    ScalarE cast pass of v1 (~81 us busy). fp16 (not bf16) because the
    logit precision from 16-bit inputs is the dominant error term:
    bf16 inputs measured rel_err 2.0e-2, right at the 2e-2 gate.
  - The softmax shift is a host-side constant 4.5*||q|| folded into the
    mask bias. Any shift cancels in the host division by Z; it only must
    keep exp() in f32 range (row max is within [2.5, 4.6]*||q|| whp for
    randn inputs). Removes v1's GPSIMD partition_all_reduce + chunk-0
    barrier. w = exp(logits) stays bf16: its exponent range matches f32,
    while fp16 w would flush to zero for plausible shifts.
  - Logits via custom DVE scans (cumsum of x*q; stride-0 output AP keeps
    each 256-element segment end; segment dots = adjacent difference of
    ends). Rows are processed in PIECES of t8-groups, sized [1,3,4] /
    [4,4] / [4,4] / [4,2,1,1]: a small first piece starts the DVE ~2.6us
    into the stream, mid pieces amortize the ~400-cycle scan op
    overhead, and the last row tapers so the post-DMA tail is one
    1-group scan (~2.7us), not a full-row scan (v2's tail was ~40us).
    Scan throughput ~1.05-1.09 cyc/elem keeps DVE (~73us) under the DMA
    stream (~83us).
  - Per piece: one [P, 8g] subtract + bias-add (DVE), one exp with
    accum_out partial-Z (ScalarE), and a burst of 4g back-to-back
    matmuls (TensorE, M=2: lhsT = two w columns [128, 2] bf16, rhs =
    their two fp16 x tiles [128, 512], one PSUM [2, 512] accumulation
    chain per row). Sustained MM bursts keep the PE HAM clock at 2.4GHz.
  - Host combines the PSUM halves and divides by Z.
"""

import numpy as np

B, N, D = 32, 8192, 256
N_CORES = 8
B_LOC = B // N_CORES  # 4
P = 128
S = 8               # consecutive positions per partition (8 KiB descriptors)
T8 = N // (P * S)   # 8 t8 groups per row
T = N // P          # 64 tiles (columns) per row

# per-row DMA/scan piece sizes in COLUMNS (1 col = one (t8, s) position =
# 256 elements = 1 KiB/partition in DRAM): small first piece to start
# compute early, and a fine taper on the last row so the post-stream tail
# is a 2-col scan (~0.7us), not a multi-group one.
ROW_PIECES = (
    (8, 24, 32),
    (32, 32),
    (32, 32),
    (16, 16, 8, 8, 8, 4, 2, 2),
)
NPIECE = sum(len(p) for p in ROW_PIECES)

_cache = {}

_SCAN_OP_NAME = "ATTNPOOL_MUL_SCAN"


def _register_scan_op():
    """Register a custom DVE op computing scan(add, Src0*Src1) in-process.

    The stock TENSOR_TENSOR_REDUCE / TENSOR_TENSOR_SCAN opcodes crash this
    terminal's ucode; custom-DVE ops ship their own uop tables inside the
    NEFF, so they are self-contained.
    """
    from concourse import dve_ops
    from concourse.dve_spec import AluOp, Spec, Src0, Src1, scan, lower, _has_src1
    from concourse.dve_uop import DveOpSpec

    for op in dve_ops.OPS:
        if op.name == _SCAN_OP_NAME:
            return op
    spec = Spec(
        body=scan(AluOp.ADD, Src0 * Src1),
        reference=lambda in0, in1, c0, c1, c2: np.cumsum(
            in0.astype(np.float32) * in1.astype(np.float32), axis=1, dtype=np.float32
        ),
    )
    row = dve_ops._CUSTOM_DVE_ROW_BASE + len(dve_ops.OPS)
    assert row < 0x20
    shas = {}
    for ver in ("v3", "v4"):
        tmp = DveOpSpec(
            name=_SCAN_OP_NAME,
            opcode=row,
            uops=lower(spec, ver=ver),
            rd1_en=_has_src1(spec),
        )
        shas[ver] = tmp.sha(ver)
    op = dve_ops.DveOp(_SCAN_OP_NAME, spec, subdim=False, uops_sha=shas)
    dve_ops.OPS.append(op)
    dve_ops._SUB_OPCODE_FOR_NAME[_SCAN_OP_NAME] = row
    dve_ops.CUSTOM_DVE_SPECS[_SCAN_OP_NAME] = spec
    return op


def _build():
    import concourse.bass as bass
    import concourse.tile as tile
    from concourse import bacc, mybir, bass_isa

    scan_op = _register_scan_op()

    dt = mybir.dt
    nc = bacc.Bacc(
        "TRN2", target_bir_lowering=False, debug=False, num_devices=N_CORES
    )
    x_d = nc.dram_tensor("x", [B_LOC, N, D], dt.float32, kind="ExternalInput").ap()
    nshift_d = nc.dram_tensor(
        "nshift", [P, 1], dt.float32, kind="ExternalInput"
    ).ap()
    q_d = nc.dram_tensor("q", [P, D], dt.float16, kind="ExternalInput").ap()
    out_d = nc.dram_tensor(
        "out", [B_LOC, 2, 2 * D], dt.float32, kind="ExternalOutput"
    ).ap()
    z_d = nc.dram_tensor("z", [P, NPIECE], dt.float32, kind="ExternalOutput").ap()

    GE = T + max(len(p) for p in ROW_PIECES)  # ends cols: zero col per piece

    with tile.TileContext(nc) as tc:
        with (
            tc.tile_pool(name="singles", bufs=1) as singles,
            tc.tile_pool(name="xrow", bufs=3) as xrow_pool,
            tc.tile_pool(name="small", bufs=2) as small,
            tc.tile_pool(name="psum", bufs=2, space="PSUM") as psum,
        ):
            qb = singles.tile([P, D], dt.float16)
            nc.sync.dma_start(qb[:], q_d[:])
            nst = singles.tile([P, 1], dt.float32)
            nc.sync.dma_start(nst[:], nshift_d[:])
            zt = singles.tile([P, NPIECE], dt.float32)
            # persistent per-row ends tiles: rows have different piece
            # layouts, so each needs its own zero-column positions. The
            # zero columns are written once here and never touched again
            # (scans only write the segment-end columns via the stride-0
            # output AP), so no per-row memset is needed.
            ends_row = [
                singles.tile([P, GE], dt.float32, name=f"ends{j}")
                for j in range(B_LOC)
            ]
            for e in ends_row:
                nc.vector.memset(e[:], 0.0)

            zcol = 0
            for b in range(B_LOC):
                pieces = ROW_PIECES[b]
                assert sum(pieces) == T
                xrow = x_d[b].rearrange("(t8 p s) d -> p t8 s d", p=P, s=S)
                rt = xrow_pool.tile([P, T, D], dt.float16)
                off = 0
                for k in pieces:
                    # DRAM side: whole-t8-group span, or a sub-group s-slice
                    if off % S == 0 and k % S == 0:
                        src = xrow[:, off // S : (off + k) // S]
                    else:
                        assert off // S == (off + k - 1) // S, (off, k)
                        src = xrow[:, off // S, off % S : off % S + k]
                    nc.gpsimd.dma_start(rt[:, off : off + k], src)
                    off += k

                ends = ends_row[b]
                logits = small.tile([P, T], dt.float32)
                w = small.tile([P, T], dt.bfloat16)
                acc = psum.tile([2, 2 * D], dt.float32)

                col0 = 0  # column offset
                ecol = 0  # ends column: zero col at ecol, ends at ecol+1..
                for k in pieces:
                    o3 = (
                        ends[:, ecol + 1 : ecol + 1 + k]
                        .rearrange("p (k u) -> p k u", u=1)
                        .broadcast_to([P, k, D])
                    )
                    nc.vector._custom_dve(
                        scan_op,
                        out=o3,
                        in0=rt[:, col0 : col0 + k],
                        in1=qb.rearrange("p (u d) -> p u d", u=1).broadcast_to(
                            [P, k, D]
                        ),
                    )
                    nc.vector.tensor_tensor(
                        logits[:, col0 : col0 + k],
                        ends[:, ecol + 1 : ecol + 1 + k],
                        ends[:, ecol : ecol + k],
                        op=mybir.AluOpType.subtract,
                    )
                    nc.scalar.activation(
                        w[:, col0 : col0 + k],
                        logits[:, col0 : col0 + k],
                        mybir.ActivationFunctionType.Exp,
                        bias=nst[:],
                        accum_out=zt[:, zcol : zcol + 1],
                    )
                    for col in range(col0, col0 + k, 2):
                        nc.tensor.matmul(
                            acc[:],
                            w[:, col : col + 2],
                            rt[:, col : col + 2].rearrange("p s d -> p (s d)"),
                            start=(col == 0),
                            stop=(col == T - 2),
                        )
                    col0 += k
                    ecol += k + 1
                    zcol += 1

                halves = small.tile([2, 2 * D], dt.float32)
                nc.scalar.copy(halves[:], acc[:])
                nc.sync.dma_start(out_d[b], halves[:])
            nc.scalar.dma_start(z_d[:], zt[:])

    nc.compile()
    return nc


def _prep_core_inputs(x, mask, q):
    """Host-side shard prep. Returns (per-core input dicts, shift).

    Masked positions are zeroed in x itself: their logit becomes exactly 0,
    so they contribute exp(-shift) to Z (subtracted on the host) and
    nothing to the weighted sum (w * 0). This removes the device-side bias
    tensor and its per-piece add entirely.
    """
    qb = np.ascontiguousarray(
        np.broadcast_to(q[None, :], (P, D))
    ).astype(np.float16)
    shift = np.float32(4.5 * np.linalg.norm(q.astype(np.float64)))
    nshift = np.full((P, 1), -shift, dtype=np.float32)
    in_maps = []
    for i in range(N_CORES):
        sl = slice(i * B_LOC, (i + 1) * B_LOC)
        xm = x[sl] * mask[sl][:, :, None]
        in_maps.append(
            {
                "x": np.ascontiguousarray(xm, dtype=np.float32),
                "nshift": nshift,
                "q": qb,
            }
        )
    return in_maps, shift


def kernel(x, mask, q, _trace=False):
    from concourse.bass_utils import run_bass_kernel_spmd

    x = np.asarray(x, dtype=np.float32)
    mask = np.asarray(mask)
    q = np.asarray(q, dtype=np.float32)
    assert x.shape == (B, N, D) and mask.shape == (B, N) and q.shape == (D,)

    if "nc" not in _cache:
        _cache["nc"] = _build()
    nc = _cache["nc"]

    in_maps, shift = _prep_core_inputs(x, mask, q)
    res = run_bass_kernel_spmd(nc, in_maps, list(range(N_CORES)), trace=_trace)

    # piece -> row mapping for the partial-Z columns
    row_of_piece = []
    for b, pieces in enumerate(ROW_PIECES):
        row_of_piece += [b] * len(pieces)
    row_of_piece = np.array(row_of_piece)

    # each masked position contributed exp(0 - shift) to Z
    emshift = np.exp(np.float64(-shift))
    n_masked = (~mask).sum(axis=1).astype(np.float64)  # [B]

    out = np.empty((B, D), dtype=np.float32)
    for i in range(N_CORES):
        sl = slice(i * B_LOC, (i + 1) * B_LOC)
        h = res.results[i]["out"]  # [B_LOC, 2, 512] PSUM halves, unnormalized
        o = h[:, 0, 0:D] + h[:, 1, D : 2 * D]
        zp = res.results[i]["z"].astype(np.float64)  # [P, NPIECE]
        z = np.array(
            [zp[:, row_of_piece == b].sum() for b in range(B_LOC)]
        )
        z -= n_masked[sl] * emshift
        out[sl] = o / z[:, None]
    if _trace:
        return out, res
    return out


# revision 16
# speedup vs baseline: 1.0000x; 1.0000x over previous
"""AttentionPool Trainium2 kernel.

Computes, for x [B, N, D], mask [B, N], q [D]:
    logits = einsum('bnd,d->bn', x, q);  logits[~mask] = -inf
    w = softmax(logits, axis=-1)
    out = einsum('bn,bnd->bd', w, x)

Sharding: data-parallel over B across 8 NeuronCores (4 rows per core).

Position enumeration (per row): n = t8*1024 + p*8 + s, with p = SBUF
partition, s in [0,8), t8 in [0,8). Each partition reads 8 consecutive
positions = 8 KiB contiguous DRAM per (p, t8). Column col = t8*8 + s.

Design (memory-roofline targeted; trace-measured notes inline):
  - x is DMA'd via the SWDGE (gpsimd) path with an inline f32 -> fp16
    cast: HBM reads stay f32 (33.5 MB/core, the roofline; measured
    ~400-420 GB/s read-side on this part), SBUF holds fp16. This removes
    the ScalarE cast pass (~81 us busy in the f32+ScalarE-cast variant).
    fp16 (not bf16) because logit precision from 16-bit inputs is the
    dominant error term: bf16 inputs measured rel_err 2.0e-2, right at
    the 2e-2 gate; fp16 measures 1.9e-3.
  - Masked positions are zeroed in x ON THE HOST: their logit becomes
    exactly 0, so they contribute exp(-shift) to Z (subtracted on the
    host, which knows the mask count) and nothing to the weighted sum.
    No device-side mask/bias tensor at all.
  - The softmax shift is a host-side constant 4.5*||q||, applied as the
    [P, 1] bias of the exp activation. Any shift cancels in the host
    division by Z; it only must keep exp() in f32 range (row max is
    within [2.5, 4.6]*||q|| whp for randn inputs). This removes the
    GPSIMD partition_all_reduce and any cross-chunk barrier.
    w = exp(logits) stays bf16: its exponent range matches f32, while
    fp16 w would flush to zero for plausible shifts.
  - Logits via custom DVE scans (cumsum of x*q; stride-0 output AP keeps
    each 256-element segment end; segment dots = adjacent difference of
    ends, one [P, k] subtract per piece). Rows are processed in column
    PIECES (see ROW_PIECES): a small first piece starts the DVE early,
    mid pieces amortize the ~400-cycle scan op overhead, and the last
    row tapers to 2-col pieces so the post-stream tail is a ~0.7us scan
    plus the last exp/matmul, not a full-row drain. Scan throughput
    ~1.05-1.09 cyc/elem keeps DVE busy (~72us) under the DMA stream
    (~80us); DVE is the second-closest engine to critical.
  - Per piece: subtract (DVE), one exp with bias=-shift and accum_out
    partial-Z (ScalarE), and a burst of k/2 back-to-back matmuls
    (TensorE, M=2: lhsT = two w columns [128, 2] bf16, rhs = their two
    fp16 x tiles [128, 512] (mixed 16-bit operands are legal), one PSUM
    [2, 512] accumulation chain per row). Sustained MM bursts let the
    PE HAM clock reach 2.4 GHz.
  - Host combines the PSUM halves, corrects Z, and divides.

Known hazard: SDMA engine 15 occasionally runs ~15% slow (SWDGE
descriptor-ring port contention, stochastic), adding 10-20us on
afflicted runs; typical runs are ~104-115us, afflicted ~117-126us.
"""

import numpy as np

B, N, D = 32, 8192, 256
N_CORES = 8
B_LOC = B // N_CORES  # 4
P = 128
S = 8               # consecutive positions per partition (8 KiB descriptors)
T8 = N // (P * S)   # 8 t8 groups per row
T = N // P          # 64 tiles (columns) per row

# per-row DMA/scan piece sizes in COLUMNS (1 col = one (t8, s) position =
# 256 elements = 1 KiB/partition in DRAM): small first piece to start
# compute early, and a fine taper on the last row so the post-stream tail
# is a 2-col scan (~0.7us), not a multi-group one.
ROW_PIECES = (
    (8, 24, 32),
    (32, 32),
    (32, 32),
    (16, 16, 8, 8, 8, 4, 2, 2),
)
NPIECE = sum(len(p) for p in ROW_PIECES)

_cache = {}

_SCAN_OP_NAME = "ATTNPOOL_MUL_SCAN"


def _register_scan_op():
    """Register a custom DVE op computing scan(add, Src0*Src1) in-process.

    The stock TENSOR_TENSOR_REDUCE / TENSOR_TENSOR_SCAN opcodes crash this
    terminal's ucode; custom-DVE ops ship their own uop tables inside the
    NEFF, so they are self-contained.
    """
    from concourse import dve_ops
    from concourse.dve_spec import AluOp, Spec, Src0, Src1, scan, lower, _has_src1
    from concourse.dve_uop import DveOpSpec

    for op in dve_ops.OPS:
        if op.name == _SCAN_OP_NAME:
            return op
    spec = Spec(
        body=scan(AluOp.ADD, Src0 * Src1),
        reference=lambda in0, in1, c0, c1, c2: np.cumsum(
            in0.astype(np.float32) * in1.astype(np.float32), axis=1, dtype=np.float32
        ),
    )
    row = dve_ops._CUSTOM_DVE_ROW_BASE + len(dve_ops.OPS)
    assert row < 0x20
    shas = {}
    for ver in ("v3", "v4"):
        tmp = DveOpSpec(
            name=_SCAN_OP_NAME,
            opcode=row,
            uops=lower(spec, ver=ver),
            rd1_en=_has_src1(spec),
        )
        shas[ver] = tmp.sha(ver)
    op = dve_ops.DveOp(_SCAN_OP_NAME, spec, subdim=False, uops_sha=shas)
    dve_ops.OPS.append(op)
    dve_ops._SUB_OPCODE_FOR_NAME[_SCAN_OP_NAME] = row
    dve_ops.CUSTOM_DVE_SPECS[_SCAN_OP_NAME] = spec
    return op


def _build():
    import concourse.bass as bass
    import concourse.tile as tile
    from concourse import bacc, mybir, bass_isa

    scan_op = _register_scan_op()

    dt = mybir.dt
    nc = bacc.Bacc(
        "TRN2", target_bir_lowering=False, debug=False, num_devices=N_CORES
    )
    x_d = nc.dram_tensor("x", [B_LOC, N, D], dt.float32, kind="ExternalInput").ap()
    nshift_d = nc.dram_tensor(
        "nshift", [P, 1], dt.float32, kind="ExternalInput"
    ).ap()
    q_d = nc.dram_tensor("q", [P, D], dt.float16, kind="ExternalInput").ap()
    out_d = nc.dram_tensor(
        "out", [B_LOC, 2, 2 * D], dt.float32, kind="ExternalOutput"
    ).ap()
    z_d = nc.dram_tensor("z", [P, NPIECE], dt.float32, kind="ExternalOutput").ap()

    GE = T + max(len(p) for p in ROW_PIECES)  # ends cols: zero col per piece

    with tile.TileContext(nc) as tc:
        with (
            tc.tile_pool(name="singles", bufs=1) as singles,
            tc.tile_pool(name="xrow", bufs=3) as xrow_pool,
            tc.tile_pool(name="small", bufs=2) as small,
            tc.tile_pool(name="psum", bufs=2, space="PSUM") as psum,
        ):
            qb = singles.tile([P, D], dt.float16)
            nc.sync.dma_start(qb[:], q_d[:])
            nst = singles.tile([P, 1], dt.float32)
            nc.sync.dma_start(nst[:], nshift_d[:])
            zt = singles.tile([P, NPIECE], dt.float32)
            # persistent per-row ends tiles: rows have different piece
            # layouts, so each needs its own zero-column positions. The
            # zero columns are written once here and never touched again
            # (scans only write the segment-end columns via the stride-0
            # output AP), so no per-row memset is needed.
            ends_row = [
                singles.tile([P, GE], dt.float32, name=f"ends{j}")
                for j in range(B_LOC)
            ]
            for e in ends_row:
                nc.vector.memset(e[:], 0.0)

            zcol = 0
            for b in range(B_LOC):
                pieces = ROW_PIECES[b]
                assert sum(pieces) == T
                xrow = x_d[b].rearrange("(t8 p s) d -> p t8 s d", p=P, s=S)
                rt = xrow_pool.tile([P, T, D], dt.float16)
                off = 0
                for k in pieces:
                    # DRAM side: whole-t8-group span, or a sub-group s-slice
                    if off % S == 0 and k % S == 0:
                        src = xrow[:, off // S : (off + k) // S]
                    else:
                        assert off // S == (off + k - 1) // S, (off, k)
                        src = xrow[:, off // S, off % S : off % S + k]
                    nc.gpsimd.dma_start(rt[:, off : off + k], src)
                    off += k

                ends = ends_row[b]
                logits = small.tile([P, T], dt.float32)
                w = small.tile([P, T], dt.bfloat16)
                acc = psum.tile([2, 2 * D], dt.float32)

                col0 = 0  # column offset
                ecol = 0  # ends column: zero col at ecol, ends at ecol+1..
                for k in pieces:
                    o3 = (
                        ends[:, ecol + 1 : ecol + 1 + k]
                        .rearrange("p (k u) -> p k u", u=1)
                        .broadcast_to([P, k, D])
                    )
                    nc.vector._custom_dve(
                        scan_op,
                        out=o3,
                        in0=rt[:, col0 : col0 + k],
                        in1=qb.rearrange("p (u d) -> p u d", u=1).broadcast_to(
                            [P, k, D]
                        ),
                    )
                    nc.vector.tensor_tensor(
                        logits[:, col0 : col0 + k],
                        ends[:, ecol + 1 : ecol + 1 + k],
                        ends[:, ecol : ecol + k],
                        op=mybir.AluOpType.subtract,
                    )
                    nc.scalar.activation(
                        w[:, col0 : col0 + k],
                        logits[:, col0 : col0 + k],
                        mybir.ActivationFunctionType.Exp,
                        bias=nst[:],
                        accum_out=zt[:, zcol : zcol + 1],
                    )
                    for col in range(col0, col0 + k, 2):
                        nc.tensor.matmul(
                            acc[:],
                            w[:, col : col + 2],
                            rt[:, col : col + 2].rearrange("p s d -> p (s d)"),
                            start=(col == 0),
                            stop=(col == T - 2),
                        )
                    col0 += k
                    ecol += k + 1
                    zcol += 1

                halves = small.tile([2, 2 * D], dt.float32)
                nc.scalar.copy(halves[:], acc[:])
                nc.sync.dma_start(out_d[b], halves[:])
            nc.scalar.dma_start(z_d[:], zt[:])

    nc.compile()
    return nc


def _prep_core_inputs(x, mask, q):
    """Host-side shard prep. Returns (per-core input dicts, shift).

    Masked positions are zeroed in x itself: their logit becomes exactly 0,
    so they contribute exp(-shift) to Z (subtracted on the host) and
    nothing to the weighted sum (w * 0). This removes the device-side bias
    tensor and its per-piece add entirely.
    """
    qb = np.ascontiguousarray(
        np.broadcast_to(q[None, :], (P, D))
    ).astype(np.float16)
    shift = np.float32(4.5 * np.linalg.norm(q.astype(np.float64)))
    nshift = np.full((P, 1), -shift, dtype=np.float32)
    in_maps = []
    for i in range(N_CORES):
        sl = slice(i * B_LOC, (i + 1) * B_LOC)
        xm = x[sl] * mask[sl][:, :, None]
        in_maps.append(
            {
                "x": np.ascontiguousarray(xm, dtype=np.float32),
                "nshift": nshift,
                "q": qb,
            }
        )
    return in_maps, shift


def kernel(x, mask, q, _trace=False):
    from concourse.bass_utils import run_bass_kernel_spmd

    x = np.asarray(x, dtype=np.float32)
    mask = np.asarray(mask)
    q = np.asarray(q, dtype=np.float32)
    assert x.shape == (B, N, D) and mask.shape == (B, N) and q.shape == (D,)

    if "nc" not in _cache:
        _cache["nc"] = _build()
    nc = _cache["nc"]

    in_maps, shift = _prep_core_inputs(x, mask, q)
    res = run_bass_kernel_spmd(nc, in_maps, list(range(N_CORES)), trace=_trace)

    # piece -> row mapping for the partial-Z columns
    row_of_piece = []
    for b, pieces in enumerate(ROW_PIECES):
        row_of_piece += [b] * len(pieces)
    row_of_piece = np.array(row_of_piece)

    # each masked position contributed exp(0 - shift) to Z
    emshift = np.exp(np.float64(-shift))
    n_masked = (~mask).sum(axis=1).astype(np.float64)  # [B]

    out = np.empty((B, D), dtype=np.float32)
    for i in range(N_CORES):
        sl = slice(i * B_LOC, (i + 1) * B_LOC)
        h = res.results[i]["out"]  # [B_LOC, 2, 512] PSUM halves, unnormalized
        o = h[:, 0, 0:D] + h[:, 1, D : 2 * D]
        zp = res.results[i]["z"].astype(np.float64)  # [P, NPIECE]
        z = np.array(
            [zp[:, row_of_piece == b].sum() for b in range(B_LOC)]
        )
        z -= n_masked[sl] * emshift
        out[sl] = o / z[:, None]
    if _trace:
        return out, res
    return out


# revision 17
# speedup vs baseline: 1.0067x; 1.0067x over previous
"""AttentionPool Trainium2 kernel.

Computes, for x [B, N, D], mask [B, N], q [D]:
    logits = einsum('bnd,d->bn', x, q);  logits[~mask] = -inf
    w = softmax(logits, axis=-1)
    out = einsum('bn,bnd->bd', w, x)

Sharding: data-parallel over B across 8 NeuronCores (4 rows per core).

Position enumeration (per row): n = t8*1024 + p*8 + s, with p = SBUF
partition, s in [0,8), t8 in [0,8). Each partition reads 8 consecutive
positions = 8 KiB contiguous DRAM per (p, t8). Column col = t8*8 + s.

Design (memory-roofline targeted; trace-measured notes inline):
  - x is DMA'd via the SWDGE (gpsimd) path with an inline f32 -> fp16
    cast: HBM reads stay f32 (33.5 MB/core, the roofline; measured
    ~400-420 GB/s read-side on this part), SBUF holds fp16. This removes
    the ScalarE cast pass (~81 us busy in the f32+ScalarE-cast variant).
    fp16 (not bf16) because logit precision from 16-bit inputs is the
    dominant error term: bf16 inputs measured rel_err 2.0e-2, right at
    the 2e-2 gate; fp16 measures 1.9e-3.
  - Masked positions are zeroed in x ON THE HOST: their logit becomes
    exactly 0, so they contribute exp(-shift) to Z (subtracted on the
    host, which knows the mask count) and nothing to the weighted sum.
    No device-side mask/bias tensor at all.
  - The softmax shift is a host-side constant 4.5*||q||, applied as the
    [P, 1] bias of the exp activation. Any shift cancels in the host
    division by Z; it only must keep exp() in f32 range (row max is
    within [2.5, 4.6]*||q|| whp for randn inputs). This removes the
    GPSIMD partition_all_reduce and any cross-chunk barrier.
    w = exp(logits) stays bf16: its exponent range matches f32, while
    fp16 w would flush to zero for plausible shifts.
  - Logits via custom DVE scans (cumsum of x*q; stride-0 output AP keeps
    each 256-element segment end; segment dots = adjacent difference of
    ends, one [P, k] subtract per piece). Rows are processed in column
    PIECES (see ROW_PIECES): a small first piece starts the DVE early,
    mid pieces amortize the ~400-cycle scan op overhead, and the last
    row tapers to 2-col pieces so the post-stream tail is a ~0.7us scan
    plus the last exp/matmul, not a full-row drain. Scan throughput
    ~1.05-1.09 cyc/elem keeps DVE busy (~72us) under the DMA stream
    (~80us); DVE is the second-closest engine to critical.
  - Per piece: subtract (DVE), one exp with bias=-shift and accum_out
    partial-Z (ScalarE), and a burst of k/2 back-to-back matmuls
    (TensorE, M=2: lhsT = two w columns [128, 2] bf16, rhs = their two
    fp16 x tiles [128, 512] (mixed 16-bit operands are legal), one PSUM
    [2, 512] accumulation chain per row). Sustained MM bursts let the
    PE HAM clock reach 2.4 GHz.
  - Host combines the PSUM halves, corrects Z, and divides.

Known hazard: SDMA engine 15 occasionally runs ~15% slow (SWDGE
descriptor-ring port contention, stochastic), adding 10-20us on
afflicted runs; typical runs are ~104-115us, afflicted ~117-126us.
"""

import numpy as np

B, N, D = 32, 8192, 256
N_CORES = 8
B_LOC = B // N_CORES  # 4
P = 128
S = 8               # consecutive positions per partition (8 KiB descriptors)
T8 = N // (P * S)   # 8 t8 groups per row
T = N // P          # 64 tiles (columns) per row

# per-row DMA/scan piece sizes in COLUMNS (1 col = one (t8, s) position =
# 256 elements = 1 KiB/partition in DRAM): small first piece to start
# compute early, and a fine taper on the last row so the post-stream tail
# is a 2-col scan (~0.7us), not a multi-group one.
ROW_PIECES = (
    (8, 24, 32),
    (32, 32),
    (32, 32),
    (16, 16, 8, 8, 8, 4, 2, 2),
)
NPIECE = sum(len(p) for p in ROW_PIECES)

_cache = {}

_SCAN_OP_NAME = "ATTNPOOL_MUL_SCAN"


def _register_scan_op():
    """Register a custom DVE op computing scan(add, Src0*Src1) in-process.

    The stock TENSOR_TENSOR_REDUCE / TENSOR_TENSOR_SCAN opcodes crash this
    terminal's ucode; custom-DVE ops ship their own uop tables inside the
    NEFF, so they are self-contained.
    """
    from concourse import dve_ops
    from concourse.dve_spec import AluOp, Spec, Src0, Src1, scan, lower, _has_src1
    from concourse.dve_uop import DveOpSpec

    for op in dve_ops.OPS:
        if op.name == _SCAN_OP_NAME:
            return op
    spec = Spec(
        body=scan(AluOp.ADD, Src0 * Src1),
        reference=lambda in0, in1, c0, c1, c2: np.cumsum(
            in0.astype(np.float32) * in1.astype(np.float32), axis=1, dtype=np.float32
        ),
    )
    row = dve_ops._CUSTOM_DVE_ROW_BASE + len(dve_ops.OPS)
    assert row < 0x20
    shas = {}
    for ver in ("v3", "v4"):
        tmp = DveOpSpec(
            name=_SCAN_OP_NAME,
            opcode=row,
            uops=lower(spec, ver=ver),
            rd1_en=_has_src1(spec),
        )
        shas[ver] = tmp.sha(ver)
    op = dve_ops.DveOp(_SCAN_OP_NAME, spec, subdim=False, uops_sha=shas)
    dve_ops.OPS.append(op)
    dve_ops._SUB_OPCODE_FOR_NAME[_SCAN_OP_NAME] = row
    dve_ops.CUSTOM_DVE_SPECS[_SCAN_OP_NAME] = spec
    return op


def _build():
    import concourse.bass as bass
    import concourse.tile as tile
    from concourse import bacc, mybir, bass_isa

    scan_op = _register_scan_op()

    dt = mybir.dt
    nc = bacc.Bacc(
        "TRN2", target_bir_lowering=False, debug=False, num_devices=N_CORES
    )
    x_d = nc.dram_tensor("x", [B_LOC, N, D], dt.float32, kind="ExternalInput").ap()
    nshift_d = nc.dram_tensor(
        "nshift", [P, 1], dt.float32, kind="ExternalInput"
    ).ap()
    q_d = nc.dram_tensor("q", [P, D], dt.float16, kind="ExternalInput").ap()
    out_d = nc.dram_tensor(
        "out", [B_LOC, 2, 2 * D], dt.float32, kind="ExternalOutput"
    ).ap()
    z_d = nc.dram_tensor("z", [P, NPIECE], dt.float32, kind="ExternalOutput").ap()

    GE = T + max(len(p) for p in ROW_PIECES)  # ends cols: zero col per piece

    with tile.TileContext(nc) as tc:
        with (
            tc.tile_pool(name="singles", bufs=1) as singles,
            tc.tile_pool(name="xrow", bufs=4) as xrow_pool,
            tc.tile_pool(name="small", bufs=2) as small,
            tc.tile_pool(name="psum", bufs=2, space="PSUM") as psum,
        ):
            qb = singles.tile([P, D], dt.float16)
            nc.sync.dma_start(qb[:], q_d[:])
            nst = singles.tile([P, 1], dt.float32)
            nc.sync.dma_start(nst[:], nshift_d[:])
            zt = singles.tile([P, NPIECE], dt.float32)
            # persistent per-row ends tiles: rows have different piece
            # layouts, so each needs its own zero-column positions. The
            # zero columns are written once here and never touched again
            # (scans only write the segment-end columns via the stride-0
            # output AP), so no per-row memset is needed.
            ends_row = [
                singles.tile([P, GE], dt.float32, name=f"ends{j}")
                for j in range(B_LOC)
            ]
            for e in ends_row:
                nc.vector.memset(e[:], 0.0)

            zcol = 0
            for b in range(B_LOC):
                pieces = ROW_PIECES[b]
                assert sum(pieces) == T
                xrow = x_d[b].rearrange("(t8 p s) d -> p t8 s d", p=P, s=S)
                rt = xrow_pool.tile([P, T, D], dt.float16)
                off = 0
                for k in pieces:
                    # DRAM side: whole-t8-group span, or a sub-group s-slice
                    if off % S == 0 and k % S == 0:
                        src = xrow[:, off // S : (off + k) // S]
                    else:
                        assert off // S == (off + k - 1) // S, (off, k)
                        src = xrow[:, off // S, off % S : off % S + k]
                    nc.gpsimd.dma_start(rt[:, off : off + k], src)
                    off += k

                ends = ends_row[b]
                logits = small.tile([P, T], dt.float32)
                w = small.tile([P, T], dt.bfloat16)
                acc = psum.tile([2, 2 * D], dt.float32)

                col0 = 0  # column offset
                ecol = 0  # ends column: zero col at ecol, ends at ecol+1..
                for k in pieces:
                    o3 = (
                        ends[:, ecol + 1 : ecol + 1 + k]
                        .rearrange("p (k u) -> p k u", u=1)
                        .broadcast_to([P, k, D])
                    )
                    nc.vector._custom_dve(
                        scan_op,
                        out=o3,
                        in0=rt[:, col0 : col0 + k],
                        in1=qb.rearrange("p (u d) -> p u d", u=1).broadcast_to(
                            [P, k, D]
                        ),
                    )
                    nc.vector.tensor_tensor(
                        logits[:, col0 : col0 + k],
                        ends[:, ecol + 1 : ecol + 1 + k],
                        ends[:, ecol : ecol + k],
                        op=mybir.AluOpType.subtract,
                    )
                    nc.scalar.activation(
                        w[:, col0 : col0 + k],
                        logits[:, col0 : col0 + k],
                        mybir.ActivationFunctionType.Exp,
                        bias=nst[:],
                        accum_out=zt[:, zcol : zcol + 1],
                    )
                    for col in range(col0, col0 + k, 2):
                        nc.tensor.matmul(
                            acc[:],
                            w[:, col : col + 2],
                            rt[:, col : col + 2].rearrange("p s d -> p (s d)"),
                            start=(col == 0),
                            stop=(col == T - 2),
                        )
                    col0 += k
                    ecol += k + 1
                    zcol += 1

                halves = small.tile([2, 2 * D], dt.float32)
                nc.scalar.copy(halves[:], acc[:])
                nc.sync.dma_start(out_d[b], halves[:])
            nc.scalar.dma_start(z_d[:], zt[:])

    nc.compile()
    return nc


def _prep_core_inputs(x, mask, q):
    """Host-side shard prep. Returns (per-core input dicts, shift).

    Masked positions are zeroed in x itself: their logit becomes exactly 0,
    so they contribute exp(-shift) to Z (subtracted on the host) and
    nothing to the weighted sum (w * 0). This removes the device-side bias
    tensor and its per-piece add entirely.
    """
    qb = np.ascontiguousarray(
        np.broadcast_to(q[None, :], (P, D))
    ).astype(np.float16)
    shift = np.float32(4.5 * np.linalg.norm(q.astype(np.float64)))
    nshift = np.full((P, 1), -shift, dtype=np.float32)
    in_maps = []
    for i in range(N_CORES):
        sl = slice(i * B_LOC, (i + 1) * B_LOC)
        xm = x[sl] * mask[sl][:, :, None]
        in_maps.append(
            {
                "x": np.ascontiguousarray(xm, dtype=np.float32),
                "nshift": nshift,
                "q": qb,
            }
        )
    return in_maps, shift


def kernel(x, mask, q, _trace=False):
    from concourse.bass_utils import run_bass_kernel_spmd

    x = np.asarray(x, dtype=np.float32)
    mask = np.asarray(mask)
    q = np.asarray(q, dtype=np.float32)
    assert x.shape == (B, N, D) and mask.shape == (B, N) and q.shape == (D,)

    if "nc" not in _cache:
        _cache["nc"] = _build()
    nc = _cache["nc"]

    in_maps, shift = _prep_core_inputs(x, mask, q)
    res = run_bass_kernel_spmd(nc, in_maps, list(range(N_CORES)), trace=_trace)

    # piece -> row mapping for the partial-Z columns
    row_of_piece = []
    for b, pieces in enumerate(ROW_PIECES):
        row_of_piece += [b] * len(pieces)
    row_of_piece = np.array(row_of_piece)

    # each masked position contributed exp(0 - shift) to Z
    emshift = np.exp(np.float64(-shift))
    n_masked = (~mask).sum(axis=1).astype(np.float64)  # [B]

    out = np.empty((B, D), dtype=np.float32)
    for i in range(N_CORES):
        sl = slice(i * B_LOC, (i + 1) * B_LOC)
        h = res.results[i]["out"]  # [B_LOC, 2, 512] PSUM halves, unnormalized
        o = h[:, 0, 0:D] + h[:, 1, D : 2 * D]
        zp = res.results[i]["z"].astype(np.float64)  # [P, NPIECE]
        z = np.array(
            [zp[:, row_of_piece == b].sum() for b in range(B_LOC)]
        )
        z -= n_masked[sl] * emshift
        out[sl] = o / z[:, None]
    if _trace:
        return out, res
    return out
